# revision 4
# baseline (speedup 1.0000x reference)
"""Trainium2 Bass kernel for nn_CapsRoutingLayer (capsule dynamic routing).

Sharding: data-parallel over batch. 8 NeuronCores, 8 batch elements each.
Each core streams W once, builds x_hat in SBUF (bf16) via full-width
128-partition PE matmuls (block-diagonal masked-x stationary per 16-n
chunk), and runs the 3 routing iterations on-core. No collectives.

Layout: SBUF partition p = 8*nn + b (nn = n mod 16, b = batch-in-core);
n = 16*chunk + nn, chunk in [0,128). xh free layout (chunk, d, o) with o
innermost so broadcasted DVE operands keep unit stride (2x mode).

s-folds run on PE: per-chunk stationary pfz[(nn,b), b'] = delta(b=b') / z[b,n]
folds both the partition sum over nn and the softmax normalization into the
accumulating matmul chain, so the DVE only does the unnormalized exp multiply.

Self-contained: hardcodes all shapes from the problem spec.
  x: (64, 2048, 8) f32;  W: (2048, 32, 16, 8) f32  ->  v: (64, 32, 16) f32
"""

import sys

sys.path.insert(0, "/opt/trn_rl_repo")

import numpy as np
import ml_dtypes

# ---- problem sizes (hardcoded) ----
B_FULL, N, O, D, I = 64, 2048, 32, 16, 8
NCORES = 8
B = B_FULL // NCORES  # 8 batch elements per core
DO = D * O  # 512, on-chip innermost layout is (d, o)
NCHUNK = N // 16  # 128 chunks of 16 n each
GW = 4  # chunks per W DMA
GXB = 16  # chunks per xb DMA
SS = 8  # superstep width (chunks) for routing elementwise passes
N_ROUTING = 3

_NC = None


def _emit(tc, dram):
    import concourse.bass as bass
    from concourse import mybir

    nc = tc.nc
    BF = mybir.dt.bfloat16
    F32 = mybir.dt.float32
    AX = mybir.AxisListType
    ALU = mybir.AluOpType
    ACTF = mybir.ActivationFunctionType

    wt_d, xb_d, xt_d, rep8_d, maskf_d, out_d = (
        dram["wt"], dram["xb"], dram["xt"], dram["rep8"], dram["maskf"],
        dram["out"],
    )

    from contextlib import ExitStack

    ctx = ExitStack()
    const = ctx.enter_context(tc.tile_pool(name="const", bufs=1))
    persist = ctx.enter_context(tc.tile_pool(name="persist", bufs=1))

    # ---- constants / inputs resident in SBUF ----
    rep8 = const.tile([8, 128], BF)
    nc.sync.dma_start(out=rep8[:], in_=rep8_d[:])
    maskf = const.tile([128, B, NCHUNK], BF)
    nc.sync.dma_start(out=maskf[:], in_=maskf_d[:])
    xt = const.tile([128, NCHUNK, B], BF)
    nc.sync.dma_start(out=xt[:], in_=xt_d[:])

    # ---- persistent big tensors ----
    xh = persist.tile([128, NCHUNK, DO], BF)  # x_hat, 128KB/partition
    logits = persist.tile([128, NCHUNK, O], F32)  # routing logits b
    cw = persist.tile([128, NCHUNK, O], BF)  # exp(logits), unnormalized
    nc.vector.memset(logits[:], 0.0)

    # ---- phase 1: build x_hat (+ s0 on PE) ----
    with tc.tile_pool(name="s0ps_pool", bufs=1, space="PSUM") as s0pool:
        s0ps = s0pool.tile([8, DO], F32)
        wctx = ExitStack()
        bps = wctx.enter_context(
            tc.tile_pool(name="buildps", bufs=6, space="PSUM")
        )
        wpool = wctx.enter_context(tc.tile_pool(name="wpool", bufs=2))
        xbpool = wctx.enter_context(tc.tile_pool(name="xbpool", bufs=2))
        xbt = None
        for g in range(NCHUNK // GW):
            wtile = wpool.tile([128, GW, DO], BF, tag="w", name=f"w_{g}")
            nc.sync.dma_start(out=wtile[:], in_=wt_d[g])
            if (g * GW) % GXB == 0:
                xbt = xbpool.tile(
                    [128, GXB, 128], BF, tag="xb", name=f"xb_{g * GW // GXB}"
                )
                nc.sync.dma_start(out=xbt[:], in_=xb_d[g * GW // GXB])
            for j in range(GW):
                m = GW * g + j
                ps = bps.tile([128, DO], F32, tag="bps", name=f"ps_{m}")
                nc.tensor.matmul(
                    ps[:], xbt[:, m % GXB, :], wtile[:, j, :],
                    start=True, stop=True,
                )
                nc.tensor.matmul(
                    s0ps[:], xt[:, m, :], wtile[:, j, :],
                    start=(m == 0), stop=(m == NCHUNK - 1),
                    skip_group_check=True,
                )
                dst = xh[:, m, :]
                # GPSIMD cannot read PSUM; alternate ACT/DVE for the drain
                if m % 2 == 0:
                    nc.scalar.copy(dst, ps[:])
                else:
                    nc.vector.tensor_copy(dst, ps[:])
        wctx.close()  # release W/xb staging SBUF + build PSUM before routing

        # ---- routing scratch ----
        scratch = ctx.enter_context(tc.tile_pool(name="scratch", bufs=1))
        ypool = ctx.enter_context(tc.tile_pool(name="ypool", bufs=3))
        vrep = scratch.tile([128, DO], BF, tag="vrep")
        vsb = scratch.tile([8, DO], BF, tag="vsb")
        pfz = scratch.tile([128, B, NCHUNK], BF, tag="pfz")
        zt1 = scratch.tile([128, NCHUNK, D], BF, tag="zt1")
        zf = scratch.tile([128, NCHUNK, 8], F32, tag="zf")
        zrb = scratch.tile([128, NCHUNK], BF, tag="zrb")
        # squash smalls
        ssb = scratch.tile([8, DO], F32, tag="ssb")
        ssq = scratch.tile([8, DO], F32, tag="ssq")
        sq1 = scratch.tile([8, 8, O], F32, tag="sq1")
        sq2 = scratch.tile([8, 4, O], F32, tag="sq2")
        sq3 = scratch.tile([8, 2, O], F32, tag="sq3")
        n2 = scratch.tile([8, O], F32, tag="n2")
        nr = scratch.tile([8, O], F32, tag="nr")
        den = scratch.tile([8, O], F32, tag="den")
        fac = scratch.tile([8, O], F32, tag="fac")
        vout = ssq  # ssq is dead once n2 is computed; reuse for f32 output

        def softmax(it):
            # cw = exp(logits) (unnormalized; 1/z folded into pfz stationary)
            nc.scalar.activation(cw[:], logits[:], ACTF.Exp)
            c3 = cw[:]
            nc.vector.tensor_add(zt1[:], c3[:, :, 0:D], c3[:, :, D:O])
            nc.vector.tensor_add(zf[:], zt1[:, :, 0:8], zt1[:, :, 8:16])
            nc.vector.tensor_add(
                zf[:, :, 0:4], zf[:, :, 0:4], zf[:, :, 4:8]
            )
            nc.vector.tensor_add(
                zf[:, :, 0:2], zf[:, :, 0:2], zf[:, :, 2:4]
            )
            nc.vector.tensor_add(zf[:, :, 0], zf[:, :, 0], zf[:, :, 1])
            nc.vector.reciprocal(zf[:, :, 1], zf[:, :, 0])
            nc.vector.tensor_copy(zrb[:], zf[:, :, 1])
            zb = zrb[:].unsqueeze(1).broadcast_to([128, B, NCHUNK])
            nc.vector.tensor_mul(pfz[:], zb, maskf[:])

        def s_step(it):
            # s[b,do] = sum_{nn,chunk} (1/z) exp(b) xh ; PE folds partitions
            # (pfz stationary) and accumulates chunks in one PSUM group.
            sps = rps.tile([8, DO], F32, tag="sps", name=f"sps_{it}")
            for ss in range(NCHUNK // SS):
                yt = ypool.tile([128, SS, DO], BF, tag="ys", name=f"ys_{it}_{ss}")
                sl = slice(SS * ss, SS * ss + SS)
                csl = (
                    cw[:, sl, :].unsqueeze(2).broadcast_to([128, SS, D, O])
                )
                nc.vector.tensor_mul(
                    yt[:].rearrange("p s (d o) -> p s d o", d=D),
                    xh[:, sl, :].rearrange("p s (d o) -> p s d o", d=D),
                    csl,
                )
                for j in range(SS):
                    m = SS * ss + j
                    nc.tensor.matmul(
                        sps[:], pfz[:, :, m], yt[:, j, :],
                        start=(m == 0), stop=(m == NCHUNK - 1),
                        skip_group_check=True,
                    )
            return sps

        def squash(sps, it, scale):
            last = it == N_ROUTING - 1
            nc.vector.tensor_copy(ssb[:], sps[:])
            nc.vector.tensor_mul(ssq[:], ssb[:], ssb[:])
            sv3 = ssq[:].rearrange("b (d o) -> b d o", d=D)
            nc.vector.tensor_add(sq1[:], sv3[:, 0:8, :], sv3[:, 8:16, :])
            nc.vector.tensor_add(sq2[:], sq1[:, 0:4, :], sq1[:, 4:8, :])
            nc.vector.tensor_add(sq3[:], sq2[:, 0:2, :], sq2[:, 2:4, :])
            nc.vector.tensor_add(n2[:], sq3[:, 0, :], sq3[:, 1, :])
            if scale != 1.0:
                nc.vector.tensor_scalar_mul(n2[:], n2[:], scale * scale)
            # sqrt(x) = exp(0.5*ln(x)): Ln+Exp share one ACT table set with
            # softmax's Exp, avoiding Sqrt<->Exp table reloads (~2.7us each)
            nc.scalar.activation(fac[:], n2[:], ACTF.Ln)
            nc.scalar.activation(nr[:], fac[:], ACTF.Exp, scale=0.5)
            nc.vector.tensor_scalar_add(den[:], n2[:], 1.0)
            nc.vector.reciprocal(fac[:], den[:])
            nc.vector.tensor_mul(fac[:], fac[:], nr[:])
            if scale != 1.0:
                nc.vector.tensor_scalar_mul(fac[:], fac[:], scale)
            fb = fac[:].unsqueeze(1).broadcast_to([8, D, O])
            sv = ssb[:].rearrange("b (d o) -> b d o", d=D)
            nc.vector.tensor_mul(vsb[:].rearrange("b (d o) -> b d o", d=D), sv, fb)
            if last:
                nc.vector.tensor_mul(
                    vout[:].rearrange("b (d o) -> b d o", d=D), sv, fb
                )
                nc.sync.dma_start(out=out_d[:], in_=vout[:])

        def a_step(it):
            # logits[b,n,o] += sum_d v[b,(d,o)] * xh[b,n,(d,o)]
            vps = rps.tile([128, DO], F32, tag="vps", name=f"vps_{it}")
            nc.tensor.matmul(vps[:], rep8[:], vsb[:], start=True, stop=True)
            nc.vector.tensor_copy(vrep[:], vps[:])
            for ss in range(NCHUNK // SS):
                yt = ypool.tile([128, SS, DO], BF, tag="ys", name=f"ya_{it}_{ss}")
                sl = slice(SS * ss, SS * ss + SS)
                vb = vrep[:].unsqueeze(1).broadcast_to([128, SS, DO])
                nc.vector.tensor_mul(yt[:], xh[:, sl, :], vb)
                yv = yt[:].rearrange("p s (d o) -> p s d o", d=D)
                # d-fold tree, balanced DVE/Pool (Pool Multiply/Add eff 0.42)
                nc.vector.tensor_add(
                    yv[:, :, 0:8, :], yv[:, :, 0:8, :], yv[:, :, 8:16, :]
                )
                nc.gpsimd.tensor_add(
                    yv[:, :, 0:4, :], yv[:, :, 0:4, :], yv[:, :, 4:8, :]
                )
                nc.gpsimd.tensor_add(
                    yv[:, :, 0:2, :], yv[:, :, 0:2, :], yv[:, :, 2:4, :]
                )
                nc.gpsimd.tensor_add(
                    yv[:, :, 0, :], yv[:, :, 0, :], yv[:, :, 1, :]
                )
                lsl = logits[:, sl, :]
                nc.vector.tensor_add(lsl, lsl, yv[:, :, 0, :])

        with tc.tile_pool(name="routps", bufs=1, space="PSUM") as rps:
            for it in range(N_ROUTING):
                if it == 0:
                    squash(s0ps, it, scale=1.0 / O)
                else:
                    softmax(it)
                    sps = s_step(it)
                    squash(sps, it, scale=1.0)
                if it < N_ROUTING - 1:
                    a_step(it)

    ctx.close()


def build_nc():
    import concourse.bass as bass
    import concourse.tile as tile
    from concourse import bacc, mybir

    BF = mybir.dt.bfloat16
    F32 = mybir.dt.float32
    nc = bacc.Bacc(
        "TRN2",
        target_bir_lowering=False,
        debug=False,
        enable_asserts=False,
        num_devices=NCORES,
    )

    rep8_np = (
        np.arange(8)[:, None] == (np.arange(128)[None, :] % 8)
    ).astype(ml_dtypes.bfloat16)
    maskf_np = np.broadcast_to(
        ((np.arange(128)[:, None] % 8) == np.arange(8)[None, :])[:, :, None],
        (128, B, NCHUNK),
    ).astype(ml_dtypes.bfloat16)
    dram = {
        "wt": nc.dram_tensor(
            "wt", [NCHUNK // GW, 128, GW, DO], BF, kind="ExternalInput"
        ).ap(),
        "xb": nc.dram_tensor(
            "xb", [NCHUNK // GXB, 128, GXB, 128], BF, kind="ExternalInput"
        ).ap(),
        "xt": nc.dram_tensor(
            "xt", [128, NCHUNK, B], BF, kind="ExternalInput"
        ).ap(),
        "rep8": nc.inline_tensor(rep8_np, name="rep8c").ap(),
        "maskf": nc.inline_tensor(maskf_np, name="maskfc").ap(),
        "out": nc.dram_tensor("out", [B, DO], F32, kind="ExternalOutput").ap(),
    }
    with tile.TileContext(nc) as tc:
        _emit(tc, dram)
    nc.compile()
    return nc


def make_host_inputs(x, W):
    """Host-side layout prep. Returns per-core in_maps."""
    bf = ml_dtypes.bfloat16
    x = np.asarray(x, np.float32)
    W = np.asarray(W, np.float32)
    # W (N, O, D, I) -> (N, I, D, O) -> [chunk, (nn,i), (d,o)] -> GW groups
    wt = (
        np.ascontiguousarray(W.transpose(0, 3, 2, 1))
        .reshape(NCHUNK, 128, DO)
        .reshape(NCHUNK // GW, GW, 128, DO)
        .transpose(0, 2, 1, 3)
    )
    wt = np.ascontiguousarray(wt).astype(bf)
    eye16 = np.eye(16, dtype=np.float32)
    in_maps = []
    for k in range(NCORES):
        xc = x[B * k : B * k + B]  # (B, N, I)
        # xtc[chunk, (nn,i), b]
        xtc = np.ascontiguousarray(xc.transpose(1, 2, 0)).reshape(
            NCHUNK, 16, I, B
        )
        # xt[p=(nn,i), chunk, b]
        xt = np.ascontiguousarray(
            xtc.reshape(NCHUNK, 128, B).transpose(1, 0, 2)
        ).astype(bf)
        # xb[chunk, (nn',i), (nn,b)] = x[b, 16chunk+nn', i] * (nn==nn')
        xb = (
            xtc[:, :, :, None, :] * eye16[None, :, None, :, None]
        ).reshape(NCHUNK, 128, 128)
        xb = np.ascontiguousarray(
            xb.reshape(NCHUNK // GXB, GXB, 128, 128).transpose(0, 2, 1, 3)
        ).astype(bf)
        in_maps.append({"wt": wt, "xb": xb, "xt": xt})
    return in_maps


def assemble_out(core_outs):
    """core_outs[k]: (B, DO) f32 in (d, o) layout -> (64, O, D) f32."""
    outs = [
        np.asarray(o, np.float32).reshape(B, D, O).transpose(0, 2, 1)
        for o in core_outs
    ]
    return np.ascontiguousarray(np.concatenate(outs, axis=0))


def run(x, W, trace=False):
    """Build (cached), execute on 8 cores, return (out, exec_time_ns)."""
    global _NC
    from concourse.bass_utils import run_bass_kernel_spmd

    if _NC is None:
        _NC = build_nc()
    in_maps = make_host_inputs(x, W)
    res = run_bass_kernel_spmd(
        _NC, in_maps, core_ids=list(range(NCORES)), trace=trace
    )
    out = assemble_out([res.results[k]["out"] for k in range(NCORES)])
    return out, res.exec_time_ns


def kernel(x, W):
    import time

    for attempt in range(3):
        try:
            out, _ = run(x, W, trace=False)
            return out
        except Exception:
            if attempt == 2:
                raise
            time.sleep(2.0)


def prep_exec(x, W):
    """Build (cached) + jit the sharded executable with device-resident
    inputs. Returns (sharded, concat_in, concat_zeros, out_avals, n_cores)."""
    global _NC
    import jax
    from jax.sharding import Mesh, PartitionSpec, NamedSharding
    from jax.experimental.shard_map import shard_map
    from concourse import mybir
    from concourse.bass2jax import (
        _bass_exec_p,
        install_neuronx_cc_hook,
        partition_id_tensor,
    )

    if _NC is None:
        _NC = build_nc()
    nc = _NC
    install_neuronx_cc_hook()
    in_maps = make_host_inputs(x, W)
    n_cores = NCORES

    in_names, out_names, out_avals, zero_outs = [], [], [], []
    partition_name = nc.partition_id_tensor.name if nc.partition_id_tensor else None
    for alloc in nc.m.functions[0].allocations:
        if not isinstance(alloc, mybir.MemoryLocationSet):
            continue
        name = alloc.memorylocations[0].name
        if alloc.kind == "ExternalInput":
            if name != partition_name:
                in_names.append(name)
        elif alloc.kind == "ExternalOutput":
            shape = list(alloc.tensor_shape)
            dt = mybir.dt.np(alloc.dtype)
            out_avals.append(jax.core.ShapedArray(shape, dt))
            out_names.append(name)
            zero_outs.append(np.zeros(shape, dt))
    n_params = len(in_names)
    n_outs = len(out_names)
    all_in_names = list(in_names) + out_names
    if partition_name is not None:
        all_in_names.append(partition_name)

    def _body(*args):
        operands = list(args)
        if partition_name is not None:
            operands.append(partition_id_tensor())
        outs = _bass_exec_p.bind(
            *operands,
            out_avals=tuple(out_avals),
            in_names=tuple(all_in_names),
            out_names=tuple(out_names),
            lowering_input_output_aliases=(),
            sim_require_finite=True,
            sim_require_nnan=True,
            nc=nc,
        )
        return tuple(outs)

    devices = jax.devices()[:n_cores]
    mesh = Mesh(np.asarray(devices), ("core",))
    in_specs = (PartitionSpec("core"),) * (n_params + n_outs)
    out_specs = (PartitionSpec("core"),) * n_outs
    sharded = jax.jit(
        shard_map(_body, mesh=mesh, in_specs=in_specs, out_specs=out_specs,
                  check_rep=False),
        keep_unused=True,
    )
    shard = NamedSharding(mesh, PartitionSpec("core"))
    concat_in = [
        jax.device_put(
            np.concatenate([np.asarray(in_maps[c][nm]) for c in range(n_cores)], 0),
            shard,
        )
        for nm in in_names
    ]
    concat_zeros = [
        jax.device_put(
            np.zeros((n_cores * z.shape[0], *z.shape[1:]), z.dtype), shard
        )
        for z in zero_outs
    ]
    return sharded, concat_in, concat_zeros, out_avals, n_cores


def bench_hw(x, W, iters=30):
    """Repeat-execute the kernel NEFF on the 8 cores, returning
    (out, wall_times_s) with one blocking round-trip per call."""
    import time
    import jax

    sharded, concat_in, concat_zeros, out_avals, n_cores = prep_exec(x, W)
    times = []
    out_arrs = None
    for i in range(iters):
        t0 = time.perf_counter()
        out_arrs = sharded(*concat_in, *concat_zeros)
        jax.block_until_ready(out_arrs)
        times.append(time.perf_counter() - t0)
    outs = [
        np.asarray(out_arrs[0]).reshape(n_cores, *out_avals[0].shape)[c]
        for c in range(n_cores)
    ]
    return assemble_out(outs), times
